# revision 12
# baseline (speedup 1.0000x reference)
"""Banded-matrix matmul kernel for Trainium2, SPMD over 8 NeuronCores.

Problem: out[b,s,o] = sum_i x[b,s,i] * W[o,i] + bias[o] with W a 4096x4096
band matrix (bandwidth 512 -> W[o,i] != 0 iff |o-i| <= 512), given in COO
form (W_values, rows, cols) with deterministic band ordering.

Strategy:
  - Host: densify W, transpose x to [in, tokens], shard tokens 8-way
    (data parallel; W + bias replicated), pack W's band into per-output-tile
    slabs so the device only ever reads the nonzero band.
  - Device (per core): out.T[o,s] = W @ x.T computed per 128-row output tile,
    accumulating over the band's k-tiles in PSUM via TensorEngine matmuls
    (weights stationary), bias added during PSUM->SBUF drain.
  - Host: gather per-core [4096, 1024] outputs, transpose back.
"""

import sys

if "/opt/trn_rl_repo" not in sys.path:
    sys.path.insert(0, "/opt/trn_rl_repo")

import numpy as np

import concourse.bass as bass
import concourse.mybir as mybir
from concourse import tile
from concourse import bass_utils
from concourse.vector_clock import ScopedClock
from concourse.bass_utils import run_bass_kernel_spmd

# ---------------------------------------------------------------- constants
N_CORES = 8
NIN = 4096
NOUT = 4096
BW = 512
B, S = 4, 2048
TOK = B * S            # 8192 tokens
TPC = TOK // N_CORES   # 1024 tokens per core
P = 128                # partitions
NT = NOUT // P         # 32 output tiles of 128
HALF = 512             # moving-operand free size per matmul (fp32 max)

# per output tile t (rows t*128..t*128+127): band spans k in [KS[t], KE[t]) (128-units)
KS = [max(0, t - BW // P) for t in range(NT)]
KE = [min(NT, t + BW // P + 1) for t in range(NT)]
NK_TOTAL = sum(KE[t] - KS[t] for t in range(NT))  # 268

COMPUTE_DT = mybir.dt.float32r  # fp32 data, full-rate PE mode
OUT_DT = mybir.dt.float32

# ------------------------------------------------- tile drain-wait workaround
# This walrus build only accepts ONE sync wait on TPB_CTRL instructions
# (Drain/NoOp); Tile's kernel-tail drain wants one wait per outstanding
# semaphore. Spread them across single-wait NOPs on the same sequencer.
_MAXW = 1


def _split_drain_and_barrier(self, tick_clock, wait_clock):
    nc = self.nc
    probe = nc.sync.nop(nofuse=True, hint="pre_drain_waits")
    wait_clock.add_sem_waits(probe.ins, ScopedClock({None: tick_clock.global_clock}))
    si = probe.ins.sync_info
    waits = list(si.on_wait) if si is not None and si.on_wait else []
    if len(waits) > _MAXW:
        probe.ins.sync_info = mybir.SyncInfo(
            on_wait=waits[:_MAXW],
            on_update=list(si.on_update) if si.on_update else [],
        )
        for i in range(_MAXW, len(waits), _MAXW):
            extra = nc.sync.nop(nofuse=True, hint=f"pre_drain_waits_{i}")
            extra.ins.sync_info = mybir.SyncInfo(
                on_wait=waits[i : i + _MAXW], on_update=[]
            )
    drain_inst = nc.sync.drain()
    wait_clock.add_sem_waits(
        drain_inst.ins, ScopedClock({None: tick_clock.global_clock})
    )
    dsi = drain_inst.ins.sync_info
    dwaits = list(dsi.on_wait) if dsi is not None and dsi.on_wait else []
    if len(dwaits) > _MAXW:
        # the NOPs above ran earlier on the same sequencer and carried them all
        drain_inst.ins.sync_info = mybir.SyncInfo(
            on_wait=[], on_update=list(dsi.on_update) if dsi.on_update else []
        )
    nc.all_engine_barrier()
    popped = nc._tile_sem_poison_stack.pop()
    assert popped is self._sem_poison
    nc.clear_and_free_semaphores(list(self.sems.allocated().values()))
    nc.all_engine_barrier()


tile.TileContext._drain_and_barrier = _split_drain_and_barrier


def fix_multi_waits(nc: bass.Bass) -> None:
    """Split >1-sync-wait compute instructions: this walrus build allows only
    one wait slot on CTRL / matmul(4-byte LDW) instructions. Carry the extra
    waits on single-wait NOPs inserted just before, on the same engine."""
    for bb in nc.m.functions[0].blocks:
        changed = False
        new_insts = []
        for inst in bb.instructions:
            si = inst.sync_info
            waits = list(si.on_wait) if si is not None and si.on_wait else []
            if len(waits) > 1:
                for w in waits[:-1]:
                    nop = mybir.InstNoOp(
                        name=nc.get_next_instruction_name(),
                        engine=inst.engine,
                        bass_nofuse=True,
                        sync_info=mybir.SyncInfo(on_wait=[w], on_update=[]),
                    )
                    new_insts.append(nop)
                inst.sync_info = mybir.SyncInfo(
                    on_wait=[waits[-1]],
                    on_update=list(si.on_update) if si.on_update else [],
                )
                changed = True
            new_insts.append(inst)
        if changed:
            bb.instructions = new_insts

# upload_artifacts reaches an internal blob store not present here; the trace
# path only needs the local files.
bass_utils.upload_artifacts = lambda tmpdir: "local://" + tmpdir


# ---------------------------------------------------------------- device IR
def build_program() -> bass.Bass:
    nc = bass.Bass()
    xT = nc.declare_dram_parameter("xT", [NIN, TPC], COMPUTE_DT, isOutput=False)
    # flat; slab t = [P, nk*P] (partition-major, exactly the SBUF layout)
    wpack = nc.declare_dram_parameter(
        "wpack", [NK_TOTAL * P * P], COMPUTE_DT, isOutput=False
    )
    # bias host-packed as [P, NT]: bias_pk[p, t] = bias[t*128 + p]
    bias = nc.declare_dram_parameter("bias_pk", [P, NT], OUT_DT, isOutput=False)
    outT = nc.declare_dram_parameter("outT", [NOUT, TPC], OUT_DT, isOutput=True)

    with tile.TileContext(nc) as tc:
        with (
            tc.tile_pool(name="xp", bufs=1) as xp,
            tc.tile_pool(name="wp", bufs=4) as wp,
            tc.tile_pool(name="op", bufs=4) as op,
            tc.tile_pool(name="bp", bufs=1) as bp,
            tc.tile_pool(name="pp", bufs=4, space="PSUM") as pp,
        ):
            bias_sb = bp.tile([P, NT], OUT_DT)
            nc.sync.dma_start(out=bias_sb[:, :], in_=bias[:, :])

            x_tiles: list = [None] * NT
            woff = 0
            for t in range(NT):
                nk = KE[t] - KS[t]
                for k in range(KS[t], KE[t]):
                    if x_tiles[k] is None:
                        xt = xp.tile([P, TPC], COMPUTE_DT, tag=f"x{k}", name=f"xt{k}")
                        nc.sync.dma_start(
                            out=xt[:, :], in_=xT[k * P : (k + 1) * P, :]
                        )
                        x_tiles[k] = xt

                wt = wp.tile([P, nk * P], COMPUTE_DT, name=f"wt{t}", tag="wt", padded_shape=[P, 9 * P])
                nc.sync.dma_start(
                    out=wt[:, :],
                    in_=wpack[woff * P * P : (woff + nk) * P * P].rearrange(
                        "(p f) -> p f", p=P
                    ),
                )

                ps0 = pp.tile([P, HALF], mybir.dt.float32, name=f"ps0_{t}", tag="ps0")
                ps1 = pp.tile([P, HALF], mybir.dt.float32, name=f"ps1_{t}", tag="ps1")
                for j in range(nk):
                    k = KS[t] + j
                    lhsT = wt[:, j * P : (j + 1) * P]
                    nc.tensor.matmul(
                        ps0[:, :], lhsT, x_tiles[k][:, 0:HALF],
                        start=(j == 0), stop=(j == nk - 1),
                    )
                    nc.tensor.matmul(
                        ps1[:, :], lhsT, x_tiles[k][:, HALF:TPC],
                        start=(j == 0), stop=(j == nk - 1),
                    )

                ot = op.tile([P, TPC], OUT_DT, name=f"ot{t}", tag="ot")
                bias_col = bias_sb[:, t : t + 1]
                nc.vector.tensor_scalar_add(ot[:, 0:HALF], ps0[:, :], bias_col)
                nc.scalar.activation(
                    ot[:, HALF:TPC], ps1[:, :],
                    mybir.ActivationFunctionType.Identity, bias=bias_col,
                )
                nc.sync.dma_start(
                    out=outT[t * P : (t + 1) * P, :], in_=ot[:, :]
                )
                woff += nk

    fix_multi_waits(nc)
    return nc


_PROGRAM_CACHE: bass.Bass | None = None


def _program() -> bass.Bass:
    global _PROGRAM_CACHE
    if _PROGRAM_CACHE is None:
        _PROGRAM_CACHE = build_program()
    return _PROGRAM_CACHE


# --------------------------------------------------------------- host side
def _pack_weights(W_values: np.ndarray, rows: np.ndarray, cols: np.ndarray) -> np.ndarray:
    W = np.zeros((NOUT, NIN), dtype=np.float32)
    W[rows, cols] = W_values
    slabs = []
    for t in range(NT):
        # slab[p, j*P + o] = W[t*P + o, (KS[t]+j)*P + p]   (SBUF layout: k on
        # partitions, [j, o] along free) -> DMA is a clean 2D pattern
        blk = W[t * P : (t + 1) * P, KS[t] * P : KE[t] * P]  # [o, nk*P]
        nk = KE[t] - KS[t]
        slab = (
            blk.reshape(P, nk, P)        # [o, j, p]
            .transpose(2, 1, 0)          # [p, j, o]
            .reshape(P, nk * P)
        )
        slabs.append(np.ascontiguousarray(slab, dtype=np.float32).ravel())
    return np.concatenate(slabs)  # flat [NK_TOTAL*P*P]


def kernel(x, W_values, bias, rows, cols, _trace=False):
    x = np.asarray(x, dtype=np.float32)
    W_values = np.asarray(W_values, dtype=np.float32)
    bias = np.asarray(bias, dtype=np.float32)
    rows = np.asarray(rows)
    cols = np.asarray(cols)

    xT = np.ascontiguousarray(x.reshape(TOK, NIN).T)  # [NIN, TOK]
    wpack = _pack_weights(W_values, rows, cols)

    bias_pk = np.ascontiguousarray(bias.reshape(NT, P).T)
    in_maps = []
    for c in range(N_CORES):
        in_maps.append(
            {
                "xT": np.ascontiguousarray(xT[:, c * TPC : (c + 1) * TPC]),
                "wpack": wpack,
                "bias_pk": bias_pk,
            }
        )

    nc = _program()
    res = run_bass_kernel_spmd(
        nc, in_maps, core_ids=list(range(N_CORES)), trace=_trace,
        trace_cores=list(range(N_CORES)) if _trace else None,
    )

    outT_full = np.empty((NOUT, TOK), dtype=np.float32)
    for c in range(N_CORES):
        outT_full[:, c * TPC : (c + 1) * TPC] = res.results[c]["outT"]
    out = np.ascontiguousarray(outT_full.T).reshape(B, S, NOUT)

    if _trace:
        kernel.last_exec_time_ns = res.exec_time_ns
        kernel.last_results = res
    return out


# revision 13
# speedup vs baseline: 1.1975x; 1.1975x over previous
"""Banded-matrix matmul kernel for Trainium2, SPMD over 8 NeuronCores.

Problem: out[b,s,o] = sum_i x[b,s,i] * W[o,i] + bias[o] with W a 4096x4096
band matrix (bandwidth 512 -> W[o,i] != 0 iff |o-i| <= 512), given in COO
form (W_values, rows, cols) with deterministic band ordering.

Strategy:
  - Host: densify W; shard tokens 8-way (data parallel; band + bias
    replicated). All device-side tensors are host-packed partition-major so
    every DMA is a 2D pattern with 8-18KB contiguous per-partition rows
    (SDMA packet overhead amortized; the band's zero padding never moves).
  - Device (per core): out.T[o,s] = W @ x.T per 128-row output tile,
    accumulating over the band's k-tiles in PSUM via float32r TensorEngine
    matmuls (full fp32 data, 1 cycle/row at N=512), bias added during the
    PSUM->SBUF drain on DVE/ACT. x streams through a sliding window of
    4-k-tile groups; W streams in 4-output-tile groups; outputs leave in
    2-output-tile stores.
  - Host: unpack per-core [128, 32*1024] outputs back to [B, S, 4096].
"""

import sys

if "/opt/trn_rl_repo" not in sys.path:
    sys.path.insert(0, "/opt/trn_rl_repo")

import numpy as np

import concourse.bass as bass
import concourse.mybir as mybir
from concourse import tile
from concourse import bass_utils
from concourse.vector_clock import ScopedClock
from concourse.bass_utils import run_bass_kernel_spmd

# ---------------------------------------------------------------- constants
N_CORES = 8
NIN = 4096
NOUT = 4096
BW = 512
B, S = 4, 2048
TOK = B * S            # 8192 tokens
TPC = TOK // N_CORES   # 1024 tokens per core
P = 128                # partitions
NT = NOUT // P         # 32 output tiles of 128 rows
HALF = 512             # moving-operand free size per matmul (4-byte max)

XG = 4                 # k-tiles per x-group       (16KB/partition rows)
WG = 4                 # o-tiles per weight group  (<=18.4KB/partition rows)
OG = 2                 # o-tiles per output store  (8KB/partition rows)
NXG = NT // XG
NWG = NT // WG

# per output tile t: band spans k-tiles [KS[t], KE[t])
KS = [max(0, t - BW // P) for t in range(NT)]
KE = [min(NT, t + BW // P + 1) for t in range(NT)]
NK = [KE[t] - KS[t] for t in range(NT)]
# weight-group layout: group g holds o-tiles [g*WG, (g+1)*WG), each slab
# [P, nk*P] partition-major, concatenated along the free axis
WGNK = [sum(NK[g * WG + i] for i in range(WG)) for g in range(NWG)]
WGOFF = [0] * NWG
for g in range(1, NWG):
    WGOFF[g] = WGOFF[g - 1] + WGNK[g - 1]
WGNK_MAX = max(WGNK)
NK_TOTAL = sum(NK)

COMPUTE_DT = mybir.dt.float32r  # fp32 data, full-rate PE mode
OUT_DT = mybir.dt.float32

# ------------------------------------------------- walrus 1-wait workaround
_MAXW = 1


def _split_drain_and_barrier(self, tick_clock, wait_clock):
    nc = self.nc
    probe = nc.sync.nop(nofuse=True, hint="pre_drain_waits")
    wait_clock.add_sem_waits(probe.ins, ScopedClock({None: tick_clock.global_clock}))
    si = probe.ins.sync_info
    waits = list(si.on_wait) if si is not None and si.on_wait else []
    if len(waits) > _MAXW:
        probe.ins.sync_info = mybir.SyncInfo(
            on_wait=waits[:_MAXW],
            on_update=list(si.on_update) if si.on_update else [],
        )
        for i in range(_MAXW, len(waits), _MAXW):
            extra = nc.sync.nop(nofuse=True, hint=f"pre_drain_waits_{i}")
            extra.ins.sync_info = mybir.SyncInfo(
                on_wait=waits[i : i + _MAXW], on_update=[]
            )
    drain_inst = nc.sync.drain()
    wait_clock.add_sem_waits(
        drain_inst.ins, ScopedClock({None: tick_clock.global_clock})
    )
    dsi = drain_inst.ins.sync_info
    dwaits = list(dsi.on_wait) if dsi is not None and dsi.on_wait else []
    if len(dwaits) > _MAXW:
        # the NOPs above ran earlier on the same sequencer and carried them all
        drain_inst.ins.sync_info = mybir.SyncInfo(
            on_wait=[], on_update=list(dsi.on_update) if dsi.on_update else []
        )
    nc.all_engine_barrier()
    popped = nc._tile_sem_poison_stack.pop()
    assert popped is self._sem_poison
    nc.clear_and_free_semaphores(list(self.sems.allocated().values()))
    nc.all_engine_barrier()


tile.TileContext._drain_and_barrier = _split_drain_and_barrier


def fix_multi_waits(nc: bass.Bass) -> None:
    """This walrus build allows only ONE sync wait per instruction. Carry
    extra waits on single-wait NOPs inserted just before, on the same
    engine/sequencer."""
    for bb in nc.m.functions[0].blocks:
        changed = False
        new_insts = []
        for inst in bb.instructions:
            si = inst.sync_info
            waits = list(si.on_wait) if si is not None and si.on_wait else []
            if len(waits) > 1:
                for w in waits[:-1]:
                    nop = mybir.InstNoOp(
                        name=nc.get_next_instruction_name(),
                        engine=inst.engine,
                        bass_nofuse=True,
                        sync_info=mybir.SyncInfo(on_wait=[w], on_update=[]),
                    )
                    new_insts.append(nop)
                inst.sync_info = mybir.SyncInfo(
                    on_wait=[waits[-1]],
                    on_update=list(si.on_update) if si.on_update else [],
                )
                changed = True
            new_insts.append(inst)
        if changed:
            bb.instructions = new_insts


# upload_artifacts reaches an internal blob store not present here; the trace
# path only needs the local files.
bass_utils.upload_artifacts = lambda tmpdir: "local://" + tmpdir


# ---------------------------------------------------------------- device IR
def build_program() -> bass.Bass:
    nc = bass.Bass()
    # all host-packed partition-major (see kernel())
    xpk = nc.declare_dram_parameter("xpk", [P, NT * TPC], COMPUTE_DT, isOutput=False)
    wpk = nc.declare_dram_parameter("wpk", [P, NK_TOTAL * P], COMPUTE_DT, isOutput=False)
    bias = nc.declare_dram_parameter("bias_pk", [P, NT], OUT_DT, isOutput=False)
    outp = nc.declare_dram_parameter("outpk", [P, NT * TPC], OUT_DT, isOutput=True)

    with tile.TileContext(nc) as tc:
        with (
            tc.tile_pool(name="xp", bufs=6) as xp,
            tc.tile_pool(name="wp", bufs=3) as wp,
            tc.tile_pool(name="op", bufs=3) as op,
            tc.tile_pool(name="bp", bufs=1) as bp,
            tc.tile_pool(name="pp", bufs=4, space="PSUM") as pp,
        ):
            bias_sb = bp.tile([P, NT], OUT_DT)
            nc.sync.dma_start(out=bias_sb[:, :], in_=bias[:, :])

            x_tiles: list = [None] * NXG
            w_tiles: list = [None] * NWG

            def load_xg(g):
                xt = xp.tile([P, XG * TPC], COMPUTE_DT, tag="xg", name=f"xg{g}")
                nc.sync.dma_start(
                    out=xt[:, :], in_=xpk[:, g * XG * TPC : (g + 1) * XG * TPC]
                )
                x_tiles[g] = xt

            def load_wg(g):
                wt = wp.tile(
                    [P, WGNK[g] * P], COMPUTE_DT, tag="wg", name=f"wg{g}",
                    padded_shape=[P, WGNK_MAX * P],
                )
                nc.sync.dma_start(
                    out=wt[:, :],
                    in_=wpk[:, WGOFF[g] * P : (WGOFF[g] + WGNK[g]) * P],
                )
                w_tiles[g] = wt

            ot = None
            for t in range(NT):
                gw = t // WG
                if w_tiles[gw] is None:
                    load_wg(gw)
                for g in range(KS[t] // XG, (KE[t] - 1) // XG + 1):
                    if x_tiles[g] is None:
                        load_xg(g)

                # slab offset of o-tile t inside its weight group
                off = sum(NK[gw * WG + i] for i in range(t - gw * WG))
                wt = w_tiles[gw]

                ps0 = pp.tile([P, HALF], mybir.dt.float32, name=f"ps0_{t}", tag="ps0")
                ps1 = pp.tile([P, HALF], mybir.dt.float32, name=f"ps1_{t}", tag="ps1")
                for j in range(NK[t]):
                    k = KS[t] + j
                    lhsT = wt[:, (off + j) * P : (off + j + 1) * P]
                    xg = x_tiles[k // XG]
                    xbase = (k % XG) * TPC
                    nc.tensor.matmul(
                        ps0[:, :], lhsT, xg[:, xbase : xbase + HALF],
                        start=(j == 0), stop=(j == NK[t] - 1),
                    )
                    nc.tensor.matmul(
                        ps1[:, :], lhsT, xg[:, xbase + HALF : xbase + TPC],
                        start=(j == 0), stop=(j == NK[t] - 1),
                    )

                if t % OG == 0:
                    ot = op.tile([P, OG * TPC], OUT_DT, name=f"ot{t}", tag="ot")
                obase = (t % OG) * TPC
                bias_col = bias_sb[:, t : t + 1]
                nc.vector.tensor_scalar_add(
                    ot[:, obase : obase + HALF], ps0[:, :], bias_col
                )
                nc.scalar.activation(
                    ot[:, obase + HALF : obase + TPC], ps1[:, :],
                    mybir.ActivationFunctionType.Identity, bias=bias_col,
                )
                if t % OG == OG - 1:
                    nc.sync.dma_start(
                        out=outp[:, (t - OG + 1) * TPC : (t + 1) * TPC],
                        in_=ot[:, :],
                    )

    fix_multi_waits(nc)
    return nc


_PROGRAM_CACHE: bass.Bass | None = None


def _program() -> bass.Bass:
    global _PROGRAM_CACHE
    if _PROGRAM_CACHE is None:
        _PROGRAM_CACHE = build_program()
    return _PROGRAM_CACHE


# --------------------------------------------------------------- host side
def _pack_weights(W_values, rows, cols) -> np.ndarray:
    W = np.zeros((NOUT, NIN), dtype=np.float32)
    W[rows, cols] = W_values
    slabs = []
    for t in range(NT):
        # slab[p, j*P + o] = W[t*P + o, (KS[t]+j)*P + p]
        blk = W[t * P : (t + 1) * P, KS[t] * P : KE[t] * P]  # [o, nk*P]
        slab = blk.reshape(P, NK[t], P).transpose(2, 1, 0).reshape(P, NK[t] * P)
        slabs.append(slab)
    return np.ascontiguousarray(np.concatenate(slabs, axis=1))  # [P, NK_TOTAL*P]


def kernel(x, W_values, bias, rows, cols, _trace=False):
    x = np.asarray(x, dtype=np.float32)
    W_values = np.asarray(W_values, dtype=np.float32)
    bias = np.asarray(bias, dtype=np.float32)
    rows = np.asarray(rows)
    cols = np.asarray(cols)

    x2d = x.reshape(TOK, NIN)
    wpk = _pack_weights(W_values, rows, cols)
    bias_pk = np.ascontiguousarray(bias.reshape(NT, P).T)

    in_maps = []
    for c in range(N_CORES):
        xs = x2d[c * TPC : (c + 1) * TPC, :]  # [TPC, NIN]
        # xpk[p, j*TPC + s] = xs[s, j*P + p]
        xpk = np.ascontiguousarray(
            xs.reshape(TPC, NT, P).transpose(2, 1, 0).reshape(P, NT * TPC)
        )
        in_maps.append({"xpk": xpk, "wpk": wpk, "bias_pk": bias_pk})

    nc = _program()
    res = run_bass_kernel_spmd(
        nc, in_maps, core_ids=list(range(N_CORES)), trace=_trace,
        trace_cores=list(range(N_CORES)) if _trace else None,
    )

    out = np.empty((TOK, NOUT), dtype=np.float32)
    for c in range(N_CORES):
        outpk = res.results[c]["outpk"]  # [P, NT*TPC]
        # out[s, t*P + p] = outpk[p, t*TPC + s]
        out[c * TPC : (c + 1) * TPC, :] = (
            outpk.reshape(P, NT, TPC).transpose(2, 1, 0).reshape(TPC, NOUT)
        )
    out = out.reshape(B, S, NOUT)

    if _trace:
        kernel.last_exec_time_ns = res.exec_time_ns
        kernel.last_results = res
    return out
